# revision 5
# baseline (speedup 1.0000x reference)
"""2-layer RNN on 8 TRN2 cores — layer-pipelined across core pairs.

Structure: 4 batch groups x 2 pipeline stages. Pair p = (core 2p, core
2p+1) handles batch group p (BL=16 sequences):
  - even core ("stage0"): layer-0 input GEMM (on x) + layer-0 recurrence;
    ships out0 chunks to its partner via pair AllGather.
  - odd core ("stage1"): layer-1 input GEMM (on received out0) + layer-1
    recurrence + FC head.

vs. the data-parallel baseline (each core: both layers, batch 8), each
weight load in the LDWEIGHTS-bound recurrence now serves 16 batch
elements instead of 8, halving per-core scan steps (512 vs 1024).

Per-core divergence uses tc.If(partition_id % 2) branches; the pair
collectives sit outside all control flow, with stage1's send skipped
(its internal send buffer ships garbage that nobody reads).
"""

import numpy as np
import ml_dtypes

B, S, I, H, C = 64, 512, 256, 512, 10
NCORES = 8
BL = B // (NCORES // 2)  # 16 sequences per pair
CH = 32                  # timesteps per shipped chunk
NCH = S // CH
NSLOT = 4                # recv ring slots

_cache = {}


def _build_nc():
    import concourse.bass as bass
    import concourse.bacc as bacc
    import concourse.tile as tile
    from concourse.bass import mybir

    f32 = mybir.dt.float32
    bf16 = mybir.dt.bfloat16
    Tanh = mybir.ActivationFunctionType.Tanh
    PAIRS = [[0, 1], [2, 3], [4, 5], [6, 7]]

    nc = bacc.Bacc("TRN2", target_bir_lowering=False, debug=False, num_devices=NCORES)

    xT_d = nc.dram_tensor("xT", [128, 2, S * BL], bf16, kind="ExternalInput")
    wL_d = nc.dram_tensor("wL", [128, 2, 4, 128], bf16, kind="ExternalInput")
    wR_d = nc.dram_tensor("wR", [128, 4, 4, 128], bf16, kind="ExternalInput")
    whh_d = nc.dram_tensor("whh", [128, 4, 4, 128], bf16, kind="ExternalInput")
    bias_d = nc.dram_tensor("bias", [128, 4], f32, kind="ExternalInput")
    wfc_d = nc.dram_tensor("wfc", [128, 4, C], bf16, kind="ExternalInput")
    bfc_d = nc.dram_tensor("bfc", [C, 1], f32, kind="ExternalInput")
    id_d = nc.dram_tensor("ident", [128, 128], bf16, kind="ExternalInput")
    out_d = nc.dram_tensor("out", [C, BL], f32, kind="ExternalOutput")

    with tile.TileContext(nc) as tc:
        with tc.tile_pool(name="sb", bufs=1) as sb, tc.tile_pool(
            name="ps", bufs=1, space="PSUM"
        ) as psp, tc.tile_pool(name="dr", bufs=1, space="DRAM") as dr:
            send_d = [
                dr.tile([128, CH, 4, BL], bf16, name=f"send{k}") for k in range(NCH)
            ]
            gath_d = [
                dr.tile([256, CH, 4, BL], bf16, name=f"gath{k}") for k in range(NCH)
            ]
            xT = sb.tile([128, 2, S * BL], bf16)
            pre = sb.tile([128, S, 4, BL], bf16)
            seq = sb.tile([128, S, 4, BL], bf16)
            recv = sb.tile([128, NSLOT, CH, 4, BL], bf16)
            wL = sb.tile([128, 2, 4, 128], bf16)
            wR = sb.tile([128, 4, 4, 128], bf16)
            whh = sb.tile([128, 4, 4, 128], bf16)
            bias = sb.tile([128, 4], f32)
            wfc = sb.tile([128, 4, C], bf16)
            bfc = sb.tile([C, 1], f32)
            ident = sb.tile([128, 128], bf16)
            fco = sb.tile([C, BL], f32)

            pid = nc.partition_id()

            # shared prologue: recurrence weights everyone needs
            nc.sync.dma_start(whh[:], whh_d[:])
            nc.sync.dma_start(bias[:], bias_d[:])
            nc.sync.dma_start(ident[:], id_d[:])
            # stage-specific loads
            with tc.If(pid % 2 == 0) as c0:
                nc.sync.dma_start(wL[:], wL_d[:])
                nc.sync.dma_start(xT[:, :, 0 : 2 * CH * BL], xT_d[:, :, 0 : 2 * CH * BL])
                nc.sync.dma_start(
                    xT[:, :, 2 * CH * BL : 8 * CH * BL],
                    xT_d[:, :, 2 * CH * BL : 8 * CH * BL],
                )
                nc.sync.dma_start(xT[:, :, 8 * CH * BL :], xT_d[:, :, 8 * CH * BL :])
            with c0.Else():
                nc.sync.dma_start(wR[:], wR_d[:])
                nc.sync.dma_start(wfc[:], wfc_d[:])
                nc.sync.dma_start(bfc[:], bfc_d[:])

            gps = [psp.tile([128, CH, BL], f32, name=f"gps{i}") for i in range(4)]
            sps = [psp.tile([128, 4, 4, BL], f32, name=f"sps{i}") for i in range(2)]

            def gemm(k):
                """Input-projection GEMM for chunk k: stage0 from x, stage1
                from the received out0 chunk. Fills pre[:, kCH:(k+1)CH]."""
                t0 = k * CH
                with tc.If(pid % 2 == 0) as c:
                    for jc in range(4):
                        ps = gps[jc]
                        for kc in range(2):
                            nc.tensor.matmul(
                                ps[:],
                                wL[:, kc, jc, :],
                                xT[:, kc, t0 * BL : (t0 + CH) * BL],
                                start=(kc == 0),
                                stop=(kc == 1),
                            )
                        nc.vector.tensor_scalar_add(
                            pre[:, t0 : t0 + CH, jc, :], ps[:], bias[:, jc : jc + 1]
                        )
                with c.Else():
                    sl = k % NSLOT
                    nc.sync.dma_start(recv[:, sl], gath_d[k][0:128])
                    for jc in range(4):
                        ps = gps[jc]
                        for kc in range(4):
                            nc.tensor.matmul(
                                ps[:],
                                wR[:, kc, jc, :],
                                recv[:, sl, :, kc, :],
                                start=(kc == 0),
                                stop=(kc == 3),
                            )
                        nc.vector.tensor_scalar_add(
                            pre[:, t0 : t0 + CH, jc, :], ps[:], bias[:, jc : jc + 1]
                        )

            def scan(t):
                ps = sps[(t // 4) % 2]
                sl = t % 4
                if sl == 0:
                    nc.tensor.matmul(
                        ps[:, 0:4, :, :],
                        ident[:],
                        pre[:, t : t + 4, :, :],
                        start=True,
                        stop=False,
                    )
                if t == 0:
                    for jc in range(4):
                        nc.scalar.activation(
                            seq[:, 0, jc, :], ps[:, 0, jc, :], Tanh
                        )
                else:
                    for jc in range(4):
                        for kc in range(4):
                            nc.tensor.matmul(
                                ps[:, sl, jc, :],
                                whh[:, kc, jc, :],
                                seq[:, t - 1, kc, :],
                                start=False,
                                stop=(kc == 3),
                            )
                        nc.scalar.activation(
                            seq[:, t, jc, :], ps[:, sl, jc, :], Tanh
                        )

            def ship(k):
                with tc.If(pid % 2 == 0):
                    nc.sync.dma_start(send_d[k][:], seq[:, k * CH : (k + 1) * CH, :, :])
                nc.gpsimd.collective_compute(
                    "AllGather",
                    mybir.AluOpType.bypass,
                    replica_groups=PAIRS,
                    ins=[send_d[k][:]],
                    outs=[gath_d[k][:]],
                )

            gemm(0)
            for t in range(S):
                scan(t)
                if (t + 1) % CH == 0:
                    k = (t + 1) // CH - 1
                    ship(k)
                    if k + 1 < NCH:
                        gemm(k + 1)

            with tc.If(pid % 2 == 1):
                fps = gps[0]
                for kc in range(4):
                    nc.tensor.matmul(
                        fps[0:C, 0, :],
                        wfc[:, kc, :],
                        seq[:, S - 1, kc, :],
                        start=(kc == 0),
                        stop=(kc == 3),
                    )
                nc.vector.tensor_scalar_add(fco[:], fps[0:C, 0, :], bfc[:])
                nc.sync.dma_start(out_d[:], fco[:])

    nc.compile()
    return nc


def _prep_inputs(inputs):
    bf = ml_dtypes.bfloat16
    f32 = np.float32

    def lhsT_4(w, n_kc):
        # w: [512, n_kc*128] -> [kp, kc, jc, jp]
        return np.ascontiguousarray(
            w.reshape(4, 128, n_kc, 128).transpose(3, 2, 0, 1)
        ).astype(bf)

    wL = lhsT_4(inputs["w_ih0"], 2)
    wR = lhsT_4(inputs["w_ih1"], 4)
    whh0 = lhsT_4(inputs["w_hh0"], 4)
    whh1 = lhsT_4(inputs["w_hh1"], 4)
    wfc = np.ascontiguousarray(
        inputs["w_fc"].reshape(C, 4, 128).transpose(2, 1, 0)
    ).astype(bf)
    b0 = np.ascontiguousarray(
        (inputs["b_ih0"] + inputs["b_hh0"]).reshape(4, 128).T
    ).astype(f32)
    b1 = np.ascontiguousarray(
        (inputs["b_ih1"] + inputs["b_hh1"]).reshape(4, 128).T
    ).astype(f32)
    bfc = inputs["b_fc"].reshape(C, 1).astype(f32)
    ident = np.eye(128, dtype=f32).astype(bf)

    zx = np.zeros((128, 2, S * BL), bf)
    zw2 = np.zeros((128, 2, 4, 128), bf)
    zw4 = np.zeros((128, 4, 4, 128), bf)
    zfc = np.zeros((128, 4, C), bf)
    zbfc = np.zeros((C, 1), f32)

    x = inputs["x"]
    in_maps = []
    for p in range(NCORES // 2):
        xs = x[p * BL : (p + 1) * BL]  # [b, t, i]
        xT = np.ascontiguousarray(
            xs.transpose(2, 1, 0).reshape(2, 128, S * BL).transpose(1, 0, 2)
        ).astype(bf)
        in_maps.append(
            {
                "xT": xT, "wL": wL, "wR": zw4, "whh": whh0, "bias": b0,
                "wfc": zfc, "bfc": zbfc, "ident": ident,
            }
        )
        in_maps.append(
            {
                "xT": zx, "wL": zw2, "wR": wR, "whh": whh1, "bias": b1,
                "wfc": wfc, "bfc": bfc, "ident": ident,
            }
        )
    return in_maps


def kernel(**inputs):
    from concourse import bass_utils

    if "nc" not in _cache:
        _cache["nc"] = _build_nc()
    nc = _cache["nc"]
    in_maps = _prep_inputs(inputs)
    res = bass_utils.run_bass_kernel_spmd(nc, in_maps, core_ids=list(range(NCORES)))
    y = np.concatenate(
        [np.asarray(res.results[2 * p + 1]["out"]).T for p in range(NCORES // 2)],
        axis=0,
    )
    return y.astype(np.float32)


# revision 19
# speedup vs baseline: 1.7197x; 1.7197x over previous
"""2-layer RNN on 8 TRN2 cores — layer-pipelined across core pairs.

Structure: 4 batch groups x 2 pipeline stages. Pair p = (core 2p, core
2p+1) handles batch group p (BL=16 sequences):
  - even core ("stage0"): layer-0 input GEMM (on x) + layer-0 recurrence;
    ships out0 chunks to its partner via pair AllGather.
  - odd core ("stage1"): layer-1 input GEMM (on received out0) + layer-1
    recurrence + FC head.

vs. the data-parallel baseline (each core: both layers, batch 8), each
weight load in the LDWEIGHTS-bound recurrence now serves 16 batch
elements instead of 8, halving per-core scan steps (512 vs 1024).

Per-core divergence uses tc.If(partition_id % 2) branches; the pair
collectives sit outside all control flow, with stage1's send skipped
(its internal send buffer ships garbage that nobody reads).
"""

import numpy as np
import ml_dtypes

B, S, I, H, C = 64, 512, 256, 512, 10
NCORES = 8
BL = B // (NCORES // 2)  # 16 sequences per pair
CH = 32                  # timesteps per shipped chunk
NCH = S // CH
NSLOT = 4                # recv ring slots

_cache = {}


def _build_nc():
    import concourse.bass as bass
    import concourse.bacc as bacc
    import concourse.tile as tile
    from concourse.bass import mybir

    import bass_rust

    f32 = mybir.dt.float32
    bf16 = mybir.dt.bfloat16
    Tanh = mybir.ActivationFunctionType.Tanh
    PAIRS = [[0, 1], [2, 3], [4, 5], [6, 7]]
    add_dep = bass_rust.add_dep_helper

    nc = bacc.Bacc("TRN2", target_bir_lowering=False, debug=False, num_devices=NCORES)

    xT_d = nc.dram_tensor("xT", [128, 2, S * BL], bf16, kind="ExternalInput")
    wL_d = nc.dram_tensor("wL", [128, 2, 4, 128], bf16, kind="ExternalInput")
    wR_d = nc.dram_tensor("wR", [128, 4, 4, 128], bf16, kind="ExternalInput")
    whh_d = nc.dram_tensor("whh", [128, 4, 4, 128], bf16, kind="ExternalInput")
    bias_d = nc.dram_tensor("bias", [128, 4], f32, kind="ExternalInput")
    wfc_d = nc.dram_tensor("wfc", [128, 4, C], bf16, kind="ExternalInput")
    bfc_d = nc.dram_tensor("bfc", [C, 1], f32, kind="ExternalInput")
    id_d = nc.dram_tensor("ident", [128, 128], bf16, kind="ExternalInput")
    out_d = nc.dram_tensor("out", [C, BL], f32, kind="ExternalOutput")

    with tile.TileContext(nc) as tc:
        with tc.tile_pool(name="sb", bufs=1) as sb, tc.tile_pool(
            name="ps", bufs=1, space="PSUM"
        ) as psp, tc.tile_pool(name="dr", bufs=1, space="DRAM") as dr:
            send_d = [
                dr.tile([128, CH, 4, BL], bf16, name=f"send{k}") for k in range(NCH)
            ]
            gath_d = [
                dr.tile([256, CH, 4, BL], bf16, name=f"gath{k}") for k in range(NCH)
            ]
            xT = sb.tile([128, 2, S * BL], bf16)
            pre = sb.tile([128, S, 4, BL], bf16)
            seq = sb.tile([128, S, 4, BL], bf16)
            recv = sb.tile([128, NSLOT, CH, 4, BL], bf16)
            wL = sb.tile([128, 2, 4, 128], bf16)
            wR = sb.tile([128, 4, 4, 128], bf16)
            whh = sb.tile([128, 4, 4, 128], bf16)
            bias = sb.tile([128, 4], f32)
            wfc = sb.tile([128, 4, C], bf16)
            bfc = sb.tile([C, 1], f32)
            ident = sb.tile([128, 128], bf16)
            fco = sb.tile([C, BL], f32)

            pid = nc.partition_id()
            probe = sb.tile([1, 1, 1, 1], bf16)

            # shared prologue: recurrence weights everyone needs
            nc.sync.dma_start(whh[:], whh_d[:])
            nc.sync.dma_start(bias[:], bias_d[:])
            nc.sync.dma_start(ident[:], id_d[:])
            # stage-specific loads
            with tc.If(pid % 2 == 0) as c0:
                nc.sync.dma_start(wL[:], wL_d[:])
                nc.sync.dma_start(xT[:, :, 0 : 2 * CH * BL], xT_d[:, :, 0 : 2 * CH * BL])
                nc.sync.dma_start(
                    xT[:, :, 2 * CH * BL : 8 * CH * BL],
                    xT_d[:, :, 2 * CH * BL : 8 * CH * BL],
                )
                nc.sync.dma_start(xT[:, :, 8 * CH * BL :], xT_d[:, :, 8 * CH * BL :])
            with c0.Else():
                nc.sync.dma_start(wR[:], wR_d[:])
                nc.sync.dma_start(wfc[:], wfc_d[:])
                nc.sync.dma_start(bfc[:], bfc_d[:])

            gps = [psp.tile([128, CH, BL], f32, name=f"gps{i}") for i in range(4)]
            # Recurrence PSUM: separate tiles for the two 8-batch halves
            # (A = batch 0:8, B = 8:16), alternating per 4-step group.
            # Emitting A's and B's 16-MM blocks back-to-back hides each
            # half's tanh(PSUM)->SBUF round trip under the other's MMs.
            sps = [psp.tile([128, 4, 4, BL // 2], f32, name=f"sps{i}") for i in range(4)]

            def gemm_L(k):
                """Stage0 input GEMM for chunk k (from x)."""
                t0 = k * CH
                for jc in range(4):
                    ps = gps[jc]
                    for kc in range(2):
                        nc.tensor.matmul(
                            ps[:],
                            wL[:, kc, jc, :],
                            xT[:, kc, t0 * BL : (t0 + CH) * BL],
                            start=(kc == 0),
                            stop=(kc == 1),
                        )
                    nc.vector.tensor_scalar_add(
                        pre[:, t0 : t0 + CH, jc, :], ps[:], bias[:, jc : jc + 1]
                    )

            def gemm_R(k):
                """Stage1 input GEMM for chunk k (from received out0)."""
                t0 = k * CH
                sl = k % NSLOT
                for jc in range(4):
                    ps = gps[jc]
                    for kc in range(4):
                        nc.tensor.matmul(
                            ps[:],
                            wR[:, kc, jc, :],
                            recv[:, sl, :, kc, :],
                            start=(kc == 0),
                            stop=(kc == 3),
                        )
                    nc.vector.tensor_scalar_add(
                        pre[:, t0 : t0 + CH, jc, :], ps[:], bias[:, jc : jc + 1]
                    )

            def scan(t):
                g = (t // 4) % 2
                sl = t % 4
                hb = BL // 2
                halves = [(sps[g], 0), (sps[2 + g], hb)]
                if sl == 0:
                    for ps, b0 in halves:
                        nc.tensor.matmul(
                            ps[:, 0:4, :, :],
                            ident[:],
                            pre[:, t : t + 4, :, b0 : b0 + hb],
                            start=True,
                            stop=False,
                        )
                if t == 0:
                    for ps, b0 in halves:
                        nc.scalar.activation(
                            seq[:, 0, :, b0 : b0 + hb], ps[:, 0, :, :], Tanh
                        )
                else:
                    for ps, b0 in halves:
                        for jc in range(4):
                            for kc in range(4):
                                nc.tensor.matmul(
                                    ps[:, sl, jc, :],
                                    whh[:, kc, jc, :],
                                    seq[:, t - 1, kc, b0 : b0 + hb],
                                    start=False,
                                    stop=(kc == 3),
                                )
                        nc.scalar.activation(
                            seq[:, t, :, b0 : b0 + hb], ps[:, sl, :, :], Tanh
                        )

            # Full arm-split chunk pipeline. Tile dependencies are
            # emission-trace based, so each arm's producer->consumer order
            # must be self-consistent; the collective's DRAM accesses are
            # tracked on neither side, hence the explicit add_dep edges:
            # even gates the trigger on its send-DMA (via a gpsimd fence in
            # the same arm), odd gates its staging DMA on the collective.
            for k in range(NCH):
                with tc.If(pid % 2 == 0):
                    gemm_L(k)
                    for t in range(k * CH, (k + 1) * CH):
                        scan(t)
                    snd = nc.sync.dma_start(
                        send_d[k][:], seq[:, k * CH : (k + 1) * CH, :, :]
                    )
                    fence = nc.gpsimd.memset(probe[:], 0)
                    add_dep(fence.ins, snd.ins, True, "trigger after send lands")
                cc = nc.gpsimd.collective_compute(
                    "AllGather",
                    mybir.AluOpType.bypass,
                    replica_groups=PAIRS,
                    ins=[send_d[k][:]],
                    outs=[gath_d[k][:]],
                )
                with tc.If(pid % 2 == 1):
                    r = nc.gpsimd.dma_start(recv[:, k % NSLOT], gath_d[k][0:128])
                    add_dep(r.ins, cc.ins, True, "recv after AllGather completes")
                    gemm_R(k)
                    for t in range(k * CH, (k + 1) * CH):
                        scan(t)

            with tc.If(pid % 2 == 1):
                fps = gps[0]
                for kc in range(4):
                    nc.tensor.matmul(
                        fps[0:C, 0, :],
                        wfc[:, kc, :],
                        seq[:, S - 1, kc, :],
                        start=(kc == 0),
                        stop=(kc == 3),
                    )
                nc.vector.tensor_scalar_add(fco[:], fps[0:C, 0, :], bfc[:])
                nc.sync.dma_start(out_d[:], fco[:])

    nc.compile()
    return nc


def _prep_inputs(inputs):
    bf = ml_dtypes.bfloat16
    f32 = np.float32

    def lhsT_4(w, n_kc):
        # w: [512, n_kc*128] -> [kp, kc, jc, jp]
        return np.ascontiguousarray(
            w.reshape(4, 128, n_kc, 128).transpose(3, 2, 0, 1)
        ).astype(bf)

    wL = lhsT_4(inputs["w_ih0"], 2)
    wR = lhsT_4(inputs["w_ih1"], 4)
    whh0 = lhsT_4(inputs["w_hh0"], 4)
    whh1 = lhsT_4(inputs["w_hh1"], 4)
    wfc = np.ascontiguousarray(
        inputs["w_fc"].reshape(C, 4, 128).transpose(2, 1, 0)
    ).astype(bf)
    b0 = np.ascontiguousarray(
        (inputs["b_ih0"] + inputs["b_hh0"]).reshape(4, 128).T
    ).astype(f32)
    b1 = np.ascontiguousarray(
        (inputs["b_ih1"] + inputs["b_hh1"]).reshape(4, 128).T
    ).astype(f32)
    bfc = inputs["b_fc"].reshape(C, 1).astype(f32)
    ident = np.eye(128, dtype=f32).astype(bf)

    zx = np.zeros((128, 2, S * BL), bf)
    zw2 = np.zeros((128, 2, 4, 128), bf)
    zw4 = np.zeros((128, 4, 4, 128), bf)
    zfc = np.zeros((128, 4, C), bf)
    zbfc = np.zeros((C, 1), f32)

    x = inputs["x"]
    in_maps = []
    for p in range(NCORES // 2):
        xs = x[p * BL : (p + 1) * BL]  # [b, t, i]
        xT = np.ascontiguousarray(
            xs.transpose(2, 1, 0).reshape(2, 128, S * BL).transpose(1, 0, 2)
        ).astype(bf)
        in_maps.append(
            {
                "xT": xT, "wL": wL, "wR": zw4, "whh": whh0, "bias": b0,
                "wfc": zfc, "bfc": zbfc, "ident": ident,
            }
        )
        in_maps.append(
            {
                "xT": zx, "wL": zw2, "wR": wR, "whh": whh1, "bias": b1,
                "wfc": wfc, "bfc": bfc, "ident": ident,
            }
        )
    return in_maps


def kernel(**inputs):
    from concourse import bass_utils

    if "nc" not in _cache:
        _cache["nc"] = _build_nc()
    nc = _cache["nc"]
    in_maps = _prep_inputs(inputs)
    res = bass_utils.run_bass_kernel_spmd(nc, in_maps, core_ids=list(range(NCORES)))
    y = np.concatenate(
        [np.asarray(res.results[2 * p + 1]["out"]).T for p in range(NCORES // 2)],
        axis=0,
    )
    return y.astype(np.float32)
